# revision 1
# baseline (speedup 1.0000x reference)
"""Trainium2 Bass kernel for nn_CELoss_4896262717859.

Computes, for each query column c = idx_node[k] of a sparse adjacency matrix
(diagonal zeroed), a cross-entropy-style loss over the "lower" (r < c) and
"upper" (r > c) neighbor sets:

    contrib_side(c) = [cnt>0 and poscnt==1] * (log(sum_r m exp(out_r)) - poslogit) / cnt

All per-column quantities are sums of the form sum_r adj[r,c] * w[r] for
w in {1, pos, pos*out, exp(out)} -> computed as tensor-engine matvecs with a
triangular split, per-column for ALL N columns, then gathered at idx_node on
the host (O(N+K) combine).

Sharding: columns split into 8 slabs of 1024 (one per core). Each core reads
its [8192 x 1024] int32 slab contiguously (memory roofline), casts to bf16,
and accumulates psum[12, 1024] stats = {L,U} x {ones, pos, pl_hi, pl_lo,
e_hi, e_lo}. The core's row order is rotated by 1024*core so the diagonal
block always falls in local row-tiles 0..7 -> one NEFF serves all cores; the
L/U routing of full tiles is data-driven via zero-padded weight variants.
"""

import numpy as np
import ml_dtypes

N = 8192
K = 4096
NCORES = 8
SLAB = N // NCORES        # 1024 columns per core
P = 128                   # partition / tile edge
NT = N // P               # 64 row tiles
TPC = SLAB // P           # 8 diagonal tiles per core
NW = 6                    # weights per side
M = 2 * NW                # 12 psum partitions (L half = 0:6, U half = 6:12)
MMN = 512                 # max matmul free size

BF16 = ml_dtypes.bfloat16

_BASS_CACHE = {}


def _build_bass():
    import concourse.tile as tile
    import concourse.mybir as mybir
    from concourse import bacc

    # Bacc (not raw Bass): its compile() runs generate_event_semaphores,
    # which splits multi-sem waits — TRN2 instructions hold at most one.
    nc = bacc.Bacc("TRN2")
    adj = nc.dram_tensor("adj", [N, SLAB], mybir.dt.int32, kind="ExternalInput")
    wmat = nc.dram_tensor(
        "wmat", [P, (NT + TPC) * M], mybir.dt.bfloat16, kind="ExternalInput"
    )
    masks = nc.dram_tensor("masks", [P, 2 * P], mybir.dt.bfloat16, kind="ExternalInput")
    stats = nc.dram_tensor("stats", [M, SLAB], mybir.dt.float32, kind="ExternalOutput")

    with tile.TileContext(nc) as tc:
        with (
            tc.tile_pool(name="singles", bufs=1) as singles,
            # bufs multiple of 8 matches the 8-queue HWDGE round-robin: the
            # slot-reuse predecessor of each adj DMA lands on the SAME queue,
            # so its WAW ordering is implicit and the DMA carries a single
            # sync-wait (the DMA ISA struct has room for only one).
            tc.tile_pool(name="io", bufs=8) as io_pool,
            tc.tile_pool(name="bf", bufs=6) as bf_pool,
            tc.tile_pool(name="diag", bufs=TPC) as diag_pool,
            tc.tile_pool(name="psum", bufs=1, space="PSUM") as psum_pool,
        ):
            # issue the first two adjacency DMAs before anything else so the
            # HBM-saturated stream (the critical path) starts ~1.3us earlier;
            # the small wmat/masks loads slot in behind them.
            pre = {}
            for j in range(2):
                t = io_pool.tile([P, SLAB], mybir.dt.int32, tag="adj_i")
                nc.sync.dma_start(out=t, in_=adj[j * P : (j + 1) * P, :])
                pre[j] = t

            wsb = singles.tile([P, (NT + TPC) * M], mybir.dt.bfloat16)
            nc.sync.dma_start(out=wsb, in_=wmat[:, :])
            msb_raw = singles.tile([P, 2 * P], mybir.dt.bfloat16)
            nc.sync.dma_start(out=msb_raw, in_=masks[:, :])
            # Re-produce the masks on DVE: the DVE TensorTensor ISA struct has
            # room for a single sync-wait, so the diag-mask multiplies must
            # only ever depend on DVE-produced operands (one self-sem wait).
            msb = singles.tile([P, 2 * P], mybir.dt.bfloat16)
            nc.vector.tensor_copy(msb, msb_raw)

            # one psum tile per 512-col bank: Tile's RAW deps are whole-tile,
            # so separate tiles let bank A's copy-out overlap bank B's final
            # matmuls
            accs = [
                psum_pool.tile(
                    [M, MMN], mybir.dt.float32, tag=f"acc{b}", name=f"acc{b}"
                )
                for b in range(SLAB // MMN)
            ]

            def wv(v):
                return wsb[:, v * M : (v + 1) * M]

            # start=True zeroes the ENTIRE psum bank(s) a matmul touches, so
            # (a) every matmul stays inside one 512-col bank, (b) exactly the
            # first matmul touching each bank carries start=True.
            bank_started = [False] * (SLAB // MMN)

            def mm_seg(w, rhs_slice, a, b, stop=False):
                bank = a // MMN
                assert b <= (bank + 1) * MMN
                nc.tensor.matmul(
                    accs[bank][:, a - bank * MMN : b - bank * MMN], w, rhs_slice,
                    start=not bank_started[bank], stop=stop,
                    skip_group_check=True,
                )
                bank_started[bank] = True

            def mm(w, rhs_full, a, b, stop=False):
                while a < b:
                    e = min(b, (a // MMN + 1) * MMN)
                    mm_seg(w, rhs_full[:, a:e], a, e, stop=stop)
                    a = e

            for j in range(NT):
                last = j == NT - 1
                if j in pre:
                    adj_i = pre.pop(j)
                else:
                    adj_i = io_pool.tile([P, SLAB], mybir.dt.int32, tag="adj_i")
                    if last:
                        # split the final load so its first half (and the
                        # bank-A matmul) overlaps the second half's transfer
                        nc.sync.dma_start(
                            out=adj_i[:, 0:MMN], in_=adj[j * P :, 0:MMN]
                        )
                        nc.sync.dma_start(
                            out=adj_i[:, MMN:], in_=adj[j * P :, MMN:]
                        )
                    else:
                        nc.sync.dma_start(out=adj_i, in_=adj[j * P : (j + 1) * P, :])
                adj_b = bf_pool.tile([P, SLAB], mybir.dt.bfloat16)
                if last:
                    # fine-grained pipeline on the final tile: shortest
                    # latency from last-byte-arrival to last matmul, with
                    # the final chunk halved again to 128 cols
                    bounds = [0, 256, 512, 768, 896, SLAB]
                    for s, e in zip(bounds[:-1], bounds[1:]):
                        nc.vector.tensor_copy(adj_b[:, s:e], adj_i[:, s:e])
                        mm(wv(j), adj_b, s, e, stop=(e == SLAB))
                    continue
                nc.vector.tensor_copy(adj_b, adj_i)

                if j < TPC:
                    WL, WU = wv(j), wv(NT + j)
                    c0, c1 = j * P, (j + 1) * P
                    mlo = diag_pool.tile([P, P], mybir.dt.bfloat16)
                    nc.vector.tensor_mul(mlo, adj_b[:, c0:c1], msb[:, 0:P])
                    mup = diag_pool.tile([P, P], mybir.dt.bfloat16)
                    nc.vector.tensor_mul(mup, adj_b[:, c0:c1], msb[:, P : 2 * P])
                    # full columns left of the diag block: rows > cols -> U
                    mm(WU, adj_b, 0, c0)
                    mm_seg(WL, mlo, c0, c1)
                    mm_seg(WU, mup, c0, c1)
                    # full columns right of the diag block: rows < cols -> L
                    mm(WL, adj_b, c1, SLAB)
                else:
                    mm(wv(j), adj_b, 0, SLAB, stop=last)

            # per-bank copy-out: bank A's copy/DMA overlap the final bank-B
            # matmul (ACT reads psum bank A while PE writes bank B); bank B's
            # copy is split across ACT and DVE so the two halves run in
            # parallel on the critical tail
            out_sb = singles.tile([M, SLAB], mybir.dt.float32)
            nc.scalar.copy(out_sb[:, 0:MMN], accs[0])
            nc.sync.dma_start(out=stats[:, 0:MMN], in_=out_sb[:, 0:MMN])
            half = MMN // 2
            nc.scalar.copy(out_sb[:, MMN : MMN + half], accs[1][:, 0:half])
            nc.vector.tensor_copy(out_sb[:, MMN + half :], accs[1][:, half:])
            nc.sync.dma_start(out=stats[:, MMN:], in_=out_sb[:, MMN:])

    nc.compile()
    return nc


def _split_bf16(v):
    hi = v.astype(BF16)
    lo = (v - hi.astype(np.float64)).astype(BF16)
    return hi, lo


def _host_prep(outputs, targets):
    """Per-row weight table Wside [N, 6] bf16 and per-core inputs."""
    out = np.asarray(outputs, np.float64).reshape(-1)
    pos = (np.asarray(targets).reshape(-1) != 0).astype(np.float64)
    pl_hi, pl_lo = _split_bf16(pos * out)
    e_hi, e_lo = _split_bf16(np.exp(out))
    wside = np.stack(
        [
            np.ones(N, BF16),
            pos.astype(BF16),
            pl_hi,
            pl_lo,
            e_hi,
            e_lo,
        ],
        axis=1,
    ).astype(BF16)  # [N, 6]

    # triangular masks for the diagonal 128-block (strict)
    ri = np.arange(P)[:, None]
    ci = np.arange(P)[None, :]
    masks = np.concatenate(
        [(ri < ci).astype(BF16), (ri > ci).astype(BF16)], axis=1
    )  # [128, 256]
    return wside, np.ascontiguousarray(masks)


def _build_wmat(wside, core):
    """Per-core weight variants [128, (64+8)*12] bf16.

    Variant j (j<64): weights for local row tile j (absolute tile (8*core+j)%64).
      j < 8  -> L-only variant (diag tiles; U-only twin stored at 64+j)
      j >= 8 -> single variant, L or U half per the tile's position vs the slab
    """
    w = np.zeros((P, NT + TPC, M), dtype=BF16)
    for j in range(NT):
        t = (TPC * core + j) % NT
        rows = wside[t * P : (t + 1) * P, :]  # [128, 6]
        if j < TPC:
            w[:, j, 0:NW] = rows
            w[:, NT + j, NW:M] = rows
        elif j < NT - TPC * core:
            w[:, j, NW:M] = rows  # rows above slab columns -> U
        else:
            w[:, j, 0:NW] = rows  # wrapped rows below slab columns -> L
    return np.ascontiguousarray(w.reshape(P, (NT + TPC) * M))


def _build_shard(node_adj, core):
    """Rotated column slab [N, SLAB] int32: local row rho = (abs_row - SLAB*core) mod N."""
    c0 = SLAB * core
    cols = node_adj[:, c0 : c0 + SLAB]
    if core == 0:
        return np.ascontiguousarray(cols, dtype=np.int32)
    return np.ascontiguousarray(
        np.concatenate([cols[c0:], cols[:c0]], axis=0), dtype=np.int32
    )


def _combine(stats_list, idx_node):
    """stats_list: per-core [12, SLAB] f32 -> scalar loss (f64 math)."""
    full = np.concatenate([np.asarray(s, np.float64) for s in stats_list], axis=1)

    def side_contrib(x):
        cnt, poscnt = x[0], x[1]
        poslogit = x[2] + x[3]
        sumexp = x[4] + x[5]
        valid = (cnt > 0.5) & (np.abs(poscnt - 1.0) < 0.25)
        lse = np.log(np.where(valid, np.maximum(sumexp, 1e-300), 1.0))
        return np.where(valid, (lse - poslogit) / np.maximum(cnt, 1.0), 0.0)

    contrib = side_contrib(full[0:NW]) + side_contrib(full[NW:M])
    idx = np.asarray(idx_node).reshape(-1).astype(np.int64)
    return np.array(contrib[idx].sum(), dtype=np.float32)


def _ensure_axon_hooks_stub():
    """bass_utils imports antenv.axon_hooks when tracing is requested via
    env; the module is absent on some images. Provide a no-op stub so the
    import never crashes (hook=None -> bass_utils skips tracing)."""
    import sys
    import types

    try:
        import antenv.axon_hooks  # noqa: F401
    except ImportError:
        mod = types.ModuleType("antenv.axon_hooks")
        state = {"hook": None}
        mod.set_axon_ntff_profile_hook = lambda h: state.__setitem__("hook", h)
        mod.get_axon_ntff_profile_hook = lambda: state["hook"]
        sys.modules["antenv.axon_hooks"] = mod


def _device_stats(in_maps):
    _ensure_axon_hooks_stub()
    from concourse.bass_utils import run_bass_kernel_spmd

    if "nc" not in _BASS_CACHE:
        _BASS_CACHE["nc"] = _build_bass()
    last_exc = None
    for attempt in range(4):
        try:
            res = run_bass_kernel_spmd(
                _BASS_CACHE["nc"], in_maps, core_ids=list(range(NCORES))
            )
            return [r["stats"] for r in res.results]
        except Exception as e:  # transient NRT/accelerator hiccups
            last_exc = e
            try:
                # a fresh PJRT client usually recovers a transiently
                # "unrecoverable" accelerator; mirrors a process restart
                import jax
                import jax.extend.backend as _jeb

                jax.clear_caches()
                _jeb.clear_backends()
            except Exception:
                pass
            import time

            time.sleep(2.0 * (attempt + 1))
    raise last_exc


def _sim_stats(in_maps):
    """Numpy emulation of the device kernel (same inputs), for logic validation."""
    outs = []
    for m in in_maps:
        adj = m["adj"].astype(np.float32)
        w = m["wmat"].reshape(P, NT + TPC, M).astype(np.float32)
        msk = m["masks"].astype(np.float32)
        lowm, upm = msk[:, 0:P], msk[:, P:]
        acc = np.zeros((M, SLAB), np.float32)
        for j in range(NT):
            tile = adj[j * P : (j + 1) * P, :]
            if j < TPC:
                WL, WU = w[:, j, :], w[:, NT + j, :]
                c0, c1 = j * P, (j + 1) * P
                acc[:, :c0] += WU.T @ tile[:, :c0]
                acc[:, c0:c1] += WL.T @ (tile[:, c0:c1] * lowm)
                acc[:, c0:c1] += WU.T @ (tile[:, c0:c1] * upm)
                acc[:, c1:] += WL.T @ tile[:, c1:]
            else:
                acc += w[:, j, :].T @ tile
        outs.append(acc)
    return outs


def kernel(outputs, targets, node_adj, idx_node, _simulate=False):
    node_adj = np.asarray(node_adj)
    wside, masks = _host_prep(outputs, targets)
    in_maps = [
        {
            "adj": _build_shard(node_adj, d),
            "wmat": _build_wmat(wside, d),
            "masks": masks,
        }
        for d in range(NCORES)
    ]
    stats = _sim_stats(in_maps) if _simulate else _device_stats(in_maps)
    return _combine(stats, idx_node)



# revision 2
# speedup vs baseline: 1.5695x; 1.5695x over previous
"""Trainium2 Bass kernel for nn_CELoss_4896262717859.

Computes, for each query column c = idx_node[k] of a sparse adjacency matrix
(diagonal zeroed), a cross-entropy-style loss over the "lower" (r < c) and
"upper" (r > c) neighbor sets:

    contrib_side(c) = [cnt>0 and poscnt==1] * (log(sum_r m exp(out_r)) - poslogit) / cnt

All per-column quantities are sums of the form sum_r adj[r,c] * w[r] for
w in {1, pos, pos*out, exp(out)} -> computed as tensor-engine matvecs with a
triangular split. Only the DISTINCT columns referenced by idx_node (~3.2k of
8192) are shipped to the device; duplicates are weighted on the host during
the O(K) combine.

Sharding: core d owns absolute columns [1024d, 1024d+1024). Its distinct
columns are grouped by crossing row-tile (c//128) into 8 groups of W=64
padded slots -> a [8192 x 512] int32 slab, read contiguously (memory
roofline), cast to bf16, accumulating psum[12, 512] stats = {L,U} x {ones,
pos, pl_hi, pl_lo, e_hi, e_lo}. The core's row order is rotated by 1024*core
so each group's crossing tile always falls in local row-tiles 0..7 -> one
NEFF serves all cores; the L/U routing of full tiles is data-driven via
zero-padded weight variants, and the per-column row threshold inside the
crossing tile is a host-built strict step mask (which also zeroes the
diagonal).
"""

import numpy as np
import ml_dtypes

N = 8192
K = 4096
NCORES = 8
CRANGE = N // NCORES      # 1024 absolute columns owned per core
P = 128                   # partition / tile edge
NT = N // P               # 64 row tiles
TPC = CRANGE // P         # 8 crossing (diag) tiles per core
W = 64                    # padded column slots per crossing tile
SLAB = TPC * W            # 512 slab columns per core
NW = 6                    # weights per side
M = 2 * NW                # 12 psum partitions (L half = 0:6, U half = 6:12)
MMN = 512                 # max matmul free size / psum bank width

BF16 = ml_dtypes.bfloat16

_BASS_CACHE = {}


def _build_bass():
    import concourse.tile as tile
    import concourse.mybir as mybir
    from concourse import bacc

    # Bacc (not raw Bass): its compile() runs generate_event_semaphores,
    # which splits multi-sem waits — TRN2 instructions hold at most one.
    nc = bacc.Bacc("TRN2")
    adj = nc.dram_tensor("adj", [N, SLAB], mybir.dt.int32, kind="ExternalInput")
    wmat = nc.dram_tensor(
        "wmat", [P, (NT + TPC) * M], mybir.dt.bfloat16, kind="ExternalInput"
    )
    masks = nc.dram_tensor(
        "masks", [P, 2 * SLAB], mybir.dt.bfloat16, kind="ExternalInput"
    )
    stats = nc.dram_tensor("stats", [M, SLAB], mybir.dt.float32, kind="ExternalOutput")

    with tile.TileContext(nc) as tc:
        with (
            tc.tile_pool(name="singles", bufs=1) as singles,
            # bufs multiple of 8 matches the 8-queue HWDGE round-robin: the
            # slot-reuse predecessor of each adj DMA lands on the SAME queue,
            # so its WAW ordering is implicit and the DMA carries a single
            # sync-wait (the DMA ISA struct has room for only one).
            tc.tile_pool(name="io", bufs=8) as io_pool,
            tc.tile_pool(name="bf", bufs=6) as bf_pool,
            tc.tile_pool(name="diag", bufs=2 * TPC) as diag_pool,
            tc.tile_pool(name="psum", bufs=1, space="PSUM") as psum_pool,
        ):
            # issue the first two adjacency DMAs before anything else so the
            # HBM-saturated stream (the critical path) starts earlier; the
            # small wmat/masks loads slot in behind them.
            pre = {}
            for j in range(2):
                t = io_pool.tile([P, SLAB], mybir.dt.int32, tag="adj_i")
                nc.sync.dma_start(out=t, in_=adj[j * P : (j + 1) * P, :])
                pre[j] = t

            wsb = singles.tile([P, (NT + TPC) * M], mybir.dt.bfloat16)
            nc.sync.dma_start(out=wsb, in_=wmat[:, :])
            msb_raw = singles.tile([P, 2 * SLAB], mybir.dt.bfloat16)
            nc.sync.dma_start(out=msb_raw, in_=masks[:, :])
            # Re-produce the masks on DVE: the DVE TensorTensor ISA struct has
            # room for a single sync-wait, so the diag-mask multiplies must
            # only ever depend on DVE-produced operands (one self-sem wait).
            msb = singles.tile([P, 2 * SLAB], mybir.dt.bfloat16)
            nc.vector.tensor_copy(msb, msb_raw)

            accs = [
                psum_pool.tile(
                    [M, MMN], mybir.dt.float32, tag=f"acc{b}", name=f"acc{b}"
                )
                for b in range(SLAB // MMN)
            ]

            def wv(v):
                return wsb[:, v * M : (v + 1) * M]

            # start=True zeroes the ENTIRE psum bank(s) a matmul touches, so
            # (a) every matmul stays inside one 512-col bank, (b) exactly the
            # first matmul touching each bank carries start=True.
            bank_started = [False] * (SLAB // MMN)

            def mm_seg(w, rhs_slice, a, b, stop=False):
                bank = a // MMN
                assert b <= (bank + 1) * MMN
                nc.tensor.matmul(
                    accs[bank][:, a - bank * MMN : b - bank * MMN], w, rhs_slice,
                    start=not bank_started[bank], stop=stop,
                    skip_group_check=True,
                )
                bank_started[bank] = True

            def mm(w, rhs_full, a, b, stop=False):
                while a < b:
                    e = min(b, (a // MMN + 1) * MMN)
                    mm_seg(w, rhs_full[:, a:e], a, e, stop=stop)
                    a = e

            for j in range(NT):
                last = j == NT - 1
                if j in pre:
                    adj_i = pre.pop(j)
                else:
                    adj_i = io_pool.tile([P, SLAB], mybir.dt.int32, tag="adj_i")
                    if last:
                        # split the final load so its first half (and the
                        # leading matmuls) overlap the second half's transfer
                        nc.sync.dma_start(
                            out=adj_i[:, 0 : SLAB // 2], in_=adj[j * P :, 0 : SLAB // 2]
                        )
                        nc.sync.dma_start(
                            out=adj_i[:, SLAB // 2 :], in_=adj[j * P :, SLAB // 2 :]
                        )
                    else:
                        nc.sync.dma_start(out=adj_i, in_=adj[j * P : (j + 1) * P, :])
                adj_b = bf_pool.tile([P, SLAB], mybir.dt.bfloat16)
                if last:
                    # fine-grained pipeline on the final tile: shortest
                    # latency from last-byte-arrival to last matmul
                    bounds = [0, 128, 256, 384, 448, SLAB]
                    for s, e in zip(bounds[:-1], bounds[1:]):
                        nc.vector.tensor_copy(adj_b[:, s:e], adj_i[:, s:e])
                        mm(wv(j), adj_b, s, e, stop=(e == SLAB))
                    continue
                nc.vector.tensor_copy(adj_b, adj_i)

                if j < TPC:
                    WL, WU = wv(j), wv(NT + j)
                    c0, c1 = j * W, (j + 1) * W
                    mlo = diag_pool.tile([P, W], mybir.dt.bfloat16)
                    nc.vector.tensor_mul(mlo, adj_b[:, c0:c1], msb[:, c0:c1])
                    mup = diag_pool.tile([P, W], mybir.dt.bfloat16)
                    nc.vector.tensor_mul(
                        mup, adj_b[:, c0:c1], msb[:, SLAB + c0 : SLAB + c1]
                    )
                    # full columns left of the crossing group: crossings
                    # already passed (rows > cols) -> U
                    if c0 > 0:
                        mm(WU, adj_b, 0, c0)
                    mm_seg(WL, mlo, c0, c1)
                    mm_seg(WU, mup, c0, c1)
                    # full columns right of the crossing group: rows < cols -> L
                    mm(WL, adj_b, c1, SLAB)
                else:
                    mm(wv(j), adj_b, 0, SLAB, stop=last)

            # copy-out tail: split across ACT and DVE so the two halves run
            # in parallel right after the final matmul
            out_sb = singles.tile([M, SLAB], mybir.dt.float32)
            half = SLAB // 2
            nc.scalar.copy(out_sb[:, 0:half], accs[0][:, 0:half])
            nc.vector.tensor_copy(out_sb[:, half:], accs[0][:, half:])
            nc.sync.dma_start(out=stats[:, :], in_=out_sb[:, :])

    nc.compile()
    return nc


def _split_bf16(v):
    hi = v.astype(BF16)
    lo = (v - hi.astype(np.float64)).astype(BF16)
    return hi, lo


def _host_weights(outputs, targets):
    """Per-row weight table Wside [N, 6] bf16."""
    out = np.asarray(outputs, np.float64).reshape(-1)
    pos = (np.asarray(targets).reshape(-1) != 0).astype(np.float64)
    pl_hi, pl_lo = _split_bf16(pos * out)
    e_hi, e_lo = _split_bf16(np.exp(out))
    return np.stack(
        [np.ones(N, BF16), pos.astype(BF16), pl_hi, pl_lo, e_hi, e_lo], axis=1
    ).astype(BF16)  # [N, 6]


def _build_wmat(wside, core):
    """Per-core weight variants [128, (64+8)*12] bf16.

    Variant j (j<64): weights for local row tile j (absolute tile (8*core+j)%64).
      j < 8  -> L-only variant (crossing tiles; U-only twin stored at 64+j)
      j >= 8 -> single variant, L or U half per the tile's position vs the slab
    """
    w = np.zeros((P, NT + TPC, M), dtype=BF16)
    for j in range(NT):
        t = (TPC * core + j) % NT
        rows = wside[t * P : (t + 1) * P, :]  # [128, 6]
        if j < TPC:
            w[:, j, 0:NW] = rows
            w[:, NT + j, NW:M] = rows
        elif j < NT - TPC * core:
            w[:, j, NW:M] = rows  # rows above slab columns -> U
        else:
            w[:, j, 0:NW] = rows  # wrapped rows below slab columns -> L
    return np.ascontiguousarray(w.reshape(P, (NT + TPC) * M))


def _plan_columns(idx_node):
    """Distinct query columns -> per-core padded slot plan.

    Returns (cols[NCORES][TPC] lists of absolute column ids, counts per slot
    group, overflow list of (col, multiplicity)).
    """
    idx = np.asarray(idx_node).reshape(-1).astype(np.int64)
    dist, mult = np.unique(idx, return_counts=True)
    plan = [[[] for _ in range(TPC)] for _ in range(NCORES)]
    overflow = []
    for c, m in zip(dist, mult):
        d, g = int(c) // CRANGE, (int(c) % CRANGE) // P
        if len(plan[d][g]) < W:
            plan[d][g].append((int(c), int(m)))
        else:
            overflow.append((int(c), int(m)))
    return plan, overflow


def _build_inputs(node_adj, wside, plan):
    """Per-core in_maps: rotated padded slab, weight variants, step masks."""
    node_adj = np.asarray(node_adj)
    in_maps = []
    ri = np.arange(P)[:, None]  # [128,1]
    for d in range(NCORES):
        slab = np.zeros((N, SLAB), np.int32)
        maskL = np.zeros((P, SLAB), BF16)
        maskU = np.zeros((P, SLAB), BF16)
        c0 = CRANGE * d
        for g in range(TPC):
            entries = plan[d][g]
            if not entries:
                continue
            cols = np.array([c for c, _ in entries], np.int64)
            sl = slice(g * W, g * W + len(cols))
            gathered = node_adj[:, cols].astype(np.int32)
            # rotate rows: local row rho = (abs_row - CRANGE*d) mod N
            slab[:, sl] = np.concatenate([gathered[c0:], gathered[:c0]], axis=0)
            # strict step masks vs the crossing tile's absolute row base;
            # strictness zeroes the diagonal on both sides
            base = c0 + g * P
            maskL[:, sl] = (base + ri < cols[None, :]).astype(BF16)
            maskU[:, sl] = (base + ri > cols[None, :]).astype(BF16)
        in_maps.append(
            {
                "adj": np.ascontiguousarray(slab),
                "wmat": _build_wmat(wside, d),
                "masks": np.ascontiguousarray(np.concatenate([maskL, maskU], axis=1)),
            }
        )
    return in_maps


def _side_contrib(x):
    cnt, poscnt = x[0], x[1]
    poslogit = x[2] + x[3]
    sumexp = x[4] + x[5]
    valid = (cnt > 0.5) & (np.abs(poscnt - 1.0) < 0.25)
    lse = np.log(np.where(valid, np.maximum(sumexp, 1e-300), 1.0))
    return np.where(valid, (lse - poslogit) / np.maximum(cnt, 1.0), 0.0)


def _combine(stats_list, plan, overflow, outputs, targets, node_adj):
    """stats_list: per-core [12, SLAB] f32 -> scalar loss (f64 math)."""
    total = 0.0
    for d in range(NCORES):
        x = np.asarray(stats_list[d], np.float64)
        contrib = _side_contrib(x[0:NW]) + _side_contrib(x[NW:M])
        for g in range(TPC):
            for i, (_, m) in enumerate(plan[d][g]):
                total += m * contrib[g * W + i]
    if overflow:
        out = np.asarray(outputs, np.float64).reshape(-1)
        pos = np.asarray(targets).reshape(-1) != 0
        rows = np.arange(N)
        for c, m in overflow:
            col = (np.asarray(node_adj[:, c]).reshape(-1) != 0) & (rows != c)
            for mask in (col & (rows < c), col & (rows >= c)):
                cnt = int(mask.sum())
                pcnt = int((mask & pos).sum())
                if cnt > 0 and pcnt == 1:
                    lse = np.log(np.exp(out[mask]).sum())
                    pl = out[mask & pos].sum()
                    total += m * (lse - pl) / cnt
    return np.float32(total)


def _ensure_axon_hooks_stub():
    """bass_utils imports antenv.axon_hooks when tracing is requested via
    env; the module is absent on some images. Provide a no-op stub so the
    import never crashes (hook=None -> bass_utils skips tracing)."""
    import sys
    import types

    try:
        import antenv.axon_hooks  # noqa: F401
    except ImportError:
        mod = types.ModuleType("antenv.axon_hooks")
        state = {"hook": None}
        mod.set_axon_ntff_profile_hook = lambda h: state.__setitem__("hook", h)
        mod.get_axon_ntff_profile_hook = lambda: state["hook"]
        sys.modules["antenv.axon_hooks"] = mod


def _device_stats(in_maps):
    _ensure_axon_hooks_stub()
    from concourse.bass_utils import run_bass_kernel_spmd

    if "nc" not in _BASS_CACHE:
        _BASS_CACHE["nc"] = _build_bass()
    last_exc = None
    for attempt in range(4):
        try:
            res = run_bass_kernel_spmd(
                _BASS_CACHE["nc"], in_maps, core_ids=list(range(NCORES))
            )
            return [r["stats"] for r in res.results]
        except Exception as e:  # transient NRT/accelerator hiccups
            last_exc = e
            try:
                # a fresh PJRT client usually recovers a transiently
                # "unrecoverable" accelerator; mirrors a process restart
                import jax
                import jax.extend.backend as _jeb

                jax.clear_caches()
                _jeb.clear_backends()
            except Exception:
                pass
            import time

            time.sleep(2.0 * (attempt + 1))
    raise last_exc


def _sim_stats(in_maps):
    """Numpy emulation of the device kernel (same inputs), for logic validation."""
    outs = []
    for m in in_maps:
        adj = m["adj"].astype(np.float32)
        w = m["wmat"].reshape(P, NT + TPC, M).astype(np.float32)
        msk = m["masks"].astype(np.float32)
        lowm, upm = msk[:, 0:SLAB], msk[:, SLAB:]
        acc = np.zeros((M, SLAB), np.float32)
        for j in range(NT):
            tile = adj[j * P : (j + 1) * P, :]
            if j < TPC:
                WL, WU = w[:, j, :], w[:, NT + j, :]
                c0, c1 = j * W, (j + 1) * W
                acc[:, :c0] += WU.T @ tile[:, :c0]
                acc[:, c0:c1] += WL.T @ (tile[:, c0:c1] * lowm[:, c0:c1])
                acc[:, c0:c1] += WU.T @ (tile[:, c0:c1] * upm[:, c0:c1])
                acc[:, c1:] += WL.T @ tile[:, c1:]
            else:
                acc += w[:, j, :].T @ tile
        outs.append(acc)
    return outs


def prepare(outputs, targets, node_adj, idx_node):
    wside = _host_weights(outputs, targets)
    plan, overflow = _plan_columns(idx_node)
    in_maps = _build_inputs(node_adj, wside, plan)
    return in_maps, plan, overflow


def kernel(outputs, targets, node_adj, idx_node, _simulate=False):
    in_maps, plan, overflow = prepare(outputs, targets, node_adj, idx_node)
    stats = _sim_stats(in_maps) if _simulate else _device_stats(in_maps)
    return _combine(stats, plan, overflow, outputs, targets, node_adj)


# revision 4
# speedup vs baseline: 3.2762x; 2.0874x over previous
"""Trainium2 Bass kernel for nn_CELoss_4896262717859.

Computes, for each query column c = idx_node[k] of a sparse adjacency matrix
(diagonal zeroed), a cross-entropy-style loss over the "lower" (r < c) and
"upper" (r > c) neighbor sets:

    contrib_side(c) = [cnt>0 and poscnt==1] * (log(sum_r m exp(out_r)) - poslogit) / cnt

All per-column quantities are sums of the form sum_r adj[r,c] * w[r] for
w in {1, pos, pos*out, exp(out)} -> tensor-engine matvecs with a triangular
split. Only the DISTINCT columns referenced by idx_node (~3.2k of 8192) are
shipped; duplicates are weighted during the O(K) host combine. The adjacency
is binary by construction, so its gathered columns are shipped as fp8 (0/1
exact) -> 1 byte/element on the wire, 8x less HBM traffic than the full
int32 matrix.

Sharding: core d owns absolute columns [1024d, 1024d+1024). Its distinct
columns are grouped by crossing row-tile (c//128) into 8 groups of W=56
padded slots -> a [8192 x 448] fp8 slab (group-overflow columns, rare, are
computed on host). Rows are rotated by 1024d so each group's crossing tile
is local row-tile g in 0..7 -> one NEFF serves all cores. The slab is stored
tile-major in DRAM in PROCESSING order (full tiles first, crossing tiles
mid-stream) and loaded in a few large chunks, psum[12, 448] accumulates
{L,U} x {ones, pos, pl_hi, pl_lo, e_hi, e_lo} via fp8 matmuls with
hi/lo-split fp8 weights; the per-column row threshold inside the crossing
tile is a host-built strict step mask.
"""

import numpy as np
import ml_dtypes

N = 8192
K = 4096
NCORES = 8
CRANGE = N // NCORES      # 1024 absolute columns owned per core
P = 128                   # partition / tile edge
NT = N // P               # 64 row tiles
TPC = CRANGE // P         # 8 crossing (diag) tiles per core
W = 56                    # padded column slots per crossing tile
SLAB = TPC * W            # 448 slab columns per core
NW = 6                    # weights per side
M = 2 * NW                # 12 psum partitions (L half = 0:6, U half = 6:12)

BF16 = ml_dtypes.bfloat16
FP8 = ml_dtypes.float8_e4m3

# processing order of local row tiles: full tiles first so the head of the
# stream needs no masks, crossing tiles (0..7, with extra DVE/PE work)
# mid-stream, full tiles again at the tail
ORDER = list(range(TPC, 52)) + list(range(0, TPC)) + list(range(52, NT))
# DMA chunk sizes in tiles (over processing positions): small head pieces so
# the first tile lands fast despite queue fair-sharing, large mid chunks to
# keep the Sync engine's ~0.7us/trigger off the critical path
CHUNKS = [1, 1, 2, 4, 16, 16, 8, 8, 4, 2, 1, 1]
assert sum(CHUNKS) == NT

_BASS_CACHE = {}


def _build_bass():
    import concourse.tile as tile
    import concourse.mybir as mybir
    from concourse import bacc

    # Bacc (not raw Bass): its compile() runs generate_event_semaphores,
    # which splits multi-sem waits — TRN2 instructions hold at most one.
    nc = bacc.Bacc("TRN2")
    adj = nc.dram_tensor("adj", [P, NT * SLAB], mybir.dt.float8e4, kind="ExternalInput")
    wmat = nc.dram_tensor(
        "wmat", [P, (NT + TPC) * M], mybir.dt.float8e4, kind="ExternalInput"
    )
    masks = nc.dram_tensor(
        "masks", [P, 2 * SLAB], mybir.dt.float8e4, kind="ExternalInput"
    )
    stats = nc.dram_tensor("stats", [M, SLAB], mybir.dt.float32, kind="ExternalOutput")

    with tile.TileContext(nc) as tc:
        with (
            tc.tile_pool(name="singles", bufs=1) as singles,
            tc.tile_pool(name="diag", bufs=2 * TPC) as diag_pool,
            tc.tile_pool(name="psum", bufs=1, space="PSUM") as psum_pool,
        ):
            # all chunks are SBUF-resident (28KB/partition total) with no
            # pool reuse -> DMA triggers carry no reuse waits at all
            chunk_tiles = []
            pos = 0
            for i, sz in enumerate(CHUNKS):
                t = singles.tile(
                    [P, sz * SLAB], mybir.dt.float8e4, tag=f"chunk{i}", name=f"chunk{i}"
                )
                chunk_tiles.append((t, pos, sz))
                pos += sz
                nc.sync.dma_start(
                    out=t, in_=adj[:, pos * SLAB - sz * SLAB : pos * SLAB]
                )
                if i == 1:
                    # small weight/mask loads slot in behind the first two
                    # adjacency pieces
                    wsb = singles.tile([P, (NT + TPC) * M], mybir.dt.float8e4)
                    nc.sync.dma_start(out=wsb, in_=wmat[:, :])
                    msb_raw = singles.tile([P, 2 * SLAB], mybir.dt.float8e4)
                    nc.sync.dma_start(out=msb_raw, in_=masks[:, :])

            # Re-produce the masks on DVE: the DVE TensorTensor ISA struct has
            # room for a single sync-wait, so the diag-mask multiplies must
            # only ever depend on DVE-produced operands (one self-sem wait).
            msb = singles.tile([P, 2 * SLAB], mybir.dt.float8e4)
            nc.vector.tensor_copy(msb, msb_raw)

            acc = psum_pool.tile([M, SLAB], mybir.dt.float32, tag="acc", name="acc")

            def wv(v):
                return wsb[:, v * M : (v + 1) * M]

            # start=True zeroes the ENTIRE psum bank a matmul touches; SLAB
            # (448) fits one 512-col bank, so only the first matmul starts.
            state = {"started": False}

            def mm(w, rhs, a, b, stop=False):
                if a >= b:
                    return
                nc.tensor.matmul(
                    acc[:, a:b], w, rhs[:, a:b],
                    start=not state["started"], stop=stop,
                    skip_group_check=True,
                )
                state["started"] = True

            for t, pos0, sz in chunk_tiles:
                for k in range(sz):
                    p = pos0 + k
                    j = ORDER[p]
                    adj_s = t[:, k * SLAB : (k + 1) * SLAB]
                    last = p == NT - 1
                    if last:
                        # fine-grained tail: shortest latency from data
                        # arrival to final matmul
                        bounds = [0, 112, 224, 336, 392, SLAB]
                        for s, e in zip(bounds[:-1], bounds[1:]):
                            mm(wv(j), adj_s, s, e, stop=(e == SLAB))
                        continue
                    if j < TPC:
                        WL, WU = wv(j), wv(NT + j)
                        c0, c1 = j * W, (j + 1) * W
                        mlo = diag_pool.tile([P, W], mybir.dt.float8e4)
                        nc.vector.tensor_mul(mlo, adj_s[:, c0:c1], msb[:, c0:c1])
                        mup = diag_pool.tile([P, W], mybir.dt.float8e4)
                        nc.vector.tensor_mul(
                            mup, adj_s[:, c0:c1], msb[:, SLAB + c0 : SLAB + c1]
                        )
                        # full columns left of the crossing group: crossings
                        # already passed (rows > cols) -> U
                        if c0 > 0:
                            mm(WU, adj_s, 0, c0)
                        nc.tensor.matmul(
                            acc[:, c0:c1], WL, mlo, start=False, stop=False,
                            skip_group_check=True,
                        )
                        nc.tensor.matmul(
                            acc[:, c0:c1], WU, mup, start=False, stop=False,
                            skip_group_check=True,
                        )
                        # full columns right of the crossing group -> L
                        mm(WL, adj_s, c1, SLAB)
                    else:
                        mm(wv(j), adj_s, 0, SLAB)

            # copy-out tail split across ACT and DVE so the halves run in
            # parallel right after the final matmul
            out_sb = singles.tile([M, SLAB], mybir.dt.float32)
            half = SLAB // 2
            nc.scalar.copy(out_sb[:, 0:half], acc[:, 0:half])
            nc.vector.tensor_copy(out_sb[:, half:], acc[:, half:])
            nc.sync.dma_start(out=stats[:, :], in_=out_sb[:, :])

    nc.compile()
    return nc


def _split_fp8(v):
    """Three-term fp8 split: v ~= a + b + c with exact-representable parts."""
    a = v.astype(FP8)
    r = v - a.astype(np.float64)
    b = r.astype(FP8)
    return a, b


def _host_weights(outputs, targets):
    """Per-row weight table Wside [N, 6] fp8 (hi/lo split pairs)."""
    out = np.asarray(outputs, np.float64).reshape(-1)
    pos = (np.asarray(targets).reshape(-1) != 0).astype(np.float64)
    pl_hi, pl_lo = _split_fp8(pos * out)
    e_hi, e_lo = _split_fp8(np.exp(out))
    return np.stack(
        [np.ones(N, FP8), pos.astype(FP8), pl_hi, pl_lo, e_hi, e_lo], axis=1
    ).astype(FP8)  # [N, 6]


def _build_wmat(wside, core):
    """Per-core weight variants [128, (64+8)*12] fp8.

    Variant j (j<64): weights for local row tile j (absolute tile (8*core+j)%64).
      j < 8  -> L-only variant (crossing tiles; U-only twin stored at 64+j)
      j >= 8 -> single variant, L or U half per the tile's position vs the slab
    """
    w = np.zeros((P, NT + TPC, M), dtype=FP8)
    for j in range(NT):
        t = (TPC * core + j) % NT
        rows = wside[t * P : (t + 1) * P, :]  # [128, 6]
        if j < TPC:
            w[:, j, 0:NW] = rows
            w[:, NT + j, NW:M] = rows
        elif j < NT - TPC * core:
            w[:, j, NW:M] = rows  # rows above slab columns -> U
        else:
            w[:, j, 0:NW] = rows  # wrapped rows below slab columns -> L
    return np.ascontiguousarray(w.reshape(P, (NT + TPC) * M))


def _plan_columns(idx_node):
    """Distinct query columns -> per-core padded slot plan.

    Returns (plan[NCORES][TPC] lists of (column, multiplicity), overflow
    list of (column, multiplicity) handled on host).
    """
    idx = np.asarray(idx_node).reshape(-1).astype(np.int64)
    dist, mult = np.unique(idx, return_counts=True)
    plan = [[[] for _ in range(TPC)] for _ in range(NCORES)]
    overflow = []
    for c, m in zip(dist, mult):
        d, g = int(c) // CRANGE, (int(c) % CRANGE) // P
        if len(plan[d][g]) < W:
            plan[d][g].append((int(c), int(m)))
        else:
            overflow.append((int(c), int(m)))
    return plan, overflow


def _build_inputs(node_adj, wside, plan):
    """Per-core in_maps: tile-major rotated fp8 slab, weights, step masks."""
    node_adj = np.asarray(node_adj)
    in_maps = []
    ri = np.arange(P)[:, None]  # [128,1]
    for d in range(NCORES):
        slab = np.zeros((N, SLAB), FP8)
        maskL = np.zeros((P, SLAB), FP8)
        maskU = np.zeros((P, SLAB), FP8)
        c0 = CRANGE * d
        for g in range(TPC):
            entries = plan[d][g]
            if not entries:
                continue
            cols = np.array([c for c, _ in entries], np.int64)
            sl = slice(g * W, g * W + len(cols))
            gathered = (node_adj[:, cols] != 0).astype(FP8)
            # rotate rows: local row rho = (abs_row - CRANGE*d) mod N
            slab[:, sl] = np.concatenate([gathered[c0:], gathered[:c0]], axis=0)
            # strict step masks vs the crossing tile's absolute row base;
            # strictness zeroes the diagonal on both sides
            base = c0 + g * P
            maskL[:, sl] = (base + ri < cols[None, :]).astype(FP8)
            maskU[:, sl] = (base + ri > cols[None, :]).astype(FP8)
        # tile-major DRAM layout in processing order
        adjT = np.zeros((P, NT * SLAB), FP8)
        for p, j in enumerate(ORDER):
            adjT[:, p * SLAB : (p + 1) * SLAB] = slab[j * P : (j + 1) * P, :]
        in_maps.append(
            {
                "adj": np.ascontiguousarray(adjT),
                "wmat": _build_wmat(wside, d),
                "masks": np.ascontiguousarray(np.concatenate([maskL, maskU], axis=1)),
            }
        )
    return in_maps


def _side_contrib(x):
    cnt, poscnt = x[0], x[1]
    poslogit = x[2] + x[3]
    sumexp = x[4] + x[5]
    valid = (cnt > 0.5) & (np.abs(poscnt - 1.0) < 0.25)
    lse = np.log(np.where(valid, np.maximum(sumexp, 1e-300), 1.0))
    return np.where(valid, (lse - poslogit) / np.maximum(cnt, 1.0), 0.0)


def _combine(stats_list, plan, overflow, outputs, targets, node_adj):
    """stats_list: per-core [12, SLAB] f32 -> scalar loss (f64 math)."""
    total = 0.0
    for d in range(NCORES):
        x = np.asarray(stats_list[d], np.float64)
        contrib = _side_contrib(x[0:NW]) + _side_contrib(x[NW:M])
        for g in range(TPC):
            for i, (_, m) in enumerate(plan[d][g]):
                total += m * contrib[g * W + i]
    if overflow:
        out = np.asarray(outputs, np.float64).reshape(-1)
        pos = np.asarray(targets).reshape(-1) != 0
        rows = np.arange(N)
        for c, m in overflow:
            col = (np.asarray(node_adj[:, c]).reshape(-1) != 0) & (rows != c)
            for mask in (col & (rows < c), col & (rows >= c)):
                cnt = int(mask.sum())
                pcnt = int((mask & pos).sum())
                if cnt > 0 and pcnt == 1:
                    lse = np.log(np.exp(out[mask]).sum())
                    pl = out[mask & pos].sum()
                    total += m * (lse - pl) / cnt
    return np.float32(total)


def _ensure_axon_hooks_stub():
    """bass_utils imports antenv.axon_hooks when tracing is requested via
    env; the module is absent on some images. Provide a no-op stub so the
    import never crashes (hook=None -> bass_utils skips tracing)."""
    import sys
    import types

    try:
        import antenv.axon_hooks  # noqa: F401
    except ImportError:
        mod = types.ModuleType("antenv.axon_hooks")
        state = {"hook": None}
        mod.set_axon_ntff_profile_hook = lambda h: state.__setitem__("hook", h)
        mod.get_axon_ntff_profile_hook = lambda: state["hook"]
        sys.modules["antenv.axon_hooks"] = mod


def _device_stats(in_maps):
    _ensure_axon_hooks_stub()
    from concourse.bass_utils import run_bass_kernel_spmd

    if "nc" not in _BASS_CACHE:
        _BASS_CACHE["nc"] = _build_bass()
    last_exc = None
    for attempt in range(4):
        try:
            res = run_bass_kernel_spmd(
                _BASS_CACHE["nc"], in_maps, core_ids=list(range(NCORES))
            )
            return [r["stats"] for r in res.results]
        except Exception as e:  # transient NRT/accelerator hiccups
            last_exc = e
            try:
                # a fresh PJRT client usually recovers a transiently
                # "unrecoverable" accelerator; mirrors a process restart
                import jax
                import jax.extend.backend as _jeb

                jax.clear_caches()
                _jeb.clear_backends()
            except Exception:
                pass
            import time

            time.sleep(2.0 * (attempt + 1))
    raise last_exc


def _sim_stats(in_maps):
    """Numpy emulation of the device kernel (same inputs incl. fp8
    quantization), for logic + precision validation."""
    outs = []
    for m in in_maps:
        adjT = m["adj"].astype(np.float32)
        w = m["wmat"].reshape(P, NT + TPC, M).astype(np.float32)
        msk = m["masks"].astype(np.float32)
        acc = np.zeros((M, SLAB), np.float32)
        for p, j in enumerate(ORDER):
            tile = adjT[:, p * SLAB : (p + 1) * SLAB]
            if j < TPC:
                WL, WU = w[:, j, :], w[:, NT + j, :]
                c0, c1 = j * W, (j + 1) * W
                acc[:, :c0] += WU.T @ tile[:, :c0]
                acc[:, c0:c1] += WL.T @ (tile[:, c0:c1] * msk[:, c0:c1])
                acc[:, c0:c1] += WU.T @ (tile[:, c0:c1] * msk[:, SLAB + c0 : SLAB + c1])
                acc[:, c1:] += WL.T @ tile[:, c1:]
            else:
                acc += w[:, j, :].T @ tile
        outs.append(acc)
    return outs


def prepare(outputs, targets, node_adj, idx_node):
    wside = _host_weights(outputs, targets)
    plan, overflow = _plan_columns(idx_node)
    in_maps = _build_inputs(node_adj, wside, plan)
    return in_maps, plan, overflow


def kernel(outputs, targets, node_adj, idx_node, _simulate=False):
    in_maps, plan, overflow = prepare(outputs, targets, node_adj, idx_node)
    stats = _sim_stats(in_maps) if _simulate else _device_stats(in_maps)
    return _combine(stats, plan, overflow, outputs, targets, node_adj)


# revision 7
# speedup vs baseline: 3.9213x; 1.1969x over previous
"""Trainium2 Bass kernel for nn_CELoss_4896262717859.

Computes, for each query column c = idx_node[k] of a sparse adjacency matrix
(diagonal zeroed), a cross-entropy-style loss over the "lower" (r < c) and
"upper" (r > c) neighbor sets:

    contrib_side(c) = [cnt>0 and poscnt==1] * (log(sum_r m exp(out_r)) - poslogit) / cnt

All per-column quantities are sums of the form sum_r adj[r,c] * w[r] for
w in {1, pos, pos*out, exp(out)} -> tensor-engine matvecs with a triangular
split. Only the DISTINCT columns referenced by idx_node (~3.2k of 8192) are
shipped; duplicates are weighted during the O(K) host combine. The adjacency
is binary by construction, so its gathered columns are shipped as fp8 (0/1
exact) -> 1 byte/element on the wire, 8x less HBM traffic than the full
int32 matrix, and row-tile PAIRS are contracted in single matmuls via the
fp8 DoubleRow perf mode.

Sharding: core d owns absolute columns [1024d, 1024d+1024). Its distinct
columns are grouped by crossing row-tile (c//128) into 8 groups of W=56
padded slots -> a [8192 x 448] fp8 slab (group-overflow columns, rare, are
computed on host). Rows are rotated by 1024d so each group's crossing tile
is local row-tile g in 0..7 -> one NEFF serves all cores. The slab is stored
tile-major in DRAM in PROCESSING order (full tiles first, crossing tiles
mid-stream) and loaded in a few large chunks; psum[16, 448] accumulates
{L,U} x {ones, pos, pl_hi, pl_lo, e_hi, e_lo} (+4 pad rows); the per-column
row threshold inside a crossing tile is a host-built strict step mask.
"""

import numpy as np
import ml_dtypes

N = 8192
K = 4096
NCORES = 8
CRANGE = N // NCORES      # 1024 absolute columns owned per core
P = 128                   # partition / tile edge
NT = N // P               # 64 row tiles
NPAIR = NT // 2           # 32 row-tile pairs
TPC = CRANGE // P         # 8 crossing (diag) tiles per core
W = 56                    # padded column slots per crossing tile
SLAB = TPC * W            # 448 slab columns per core
NW = 6                    # weights per side
M = 2 * NW                # 12 psum stat rows (L half = 0:6, U half = 6:12)
MP = 16                   # padded weight width (DoubleRow needs 16B steps)

BF16 = ml_dtypes.bfloat16
FP8 = ml_dtypes.float8_e4m3

# processing order of local row tiles: full tiles first so the head of the
# stream needs no masks, crossing tiles (0..7, extra work) mid-stream
ORDER = list(range(TPC, 52)) + list(range(0, TPC)) + list(range(52, NT))
CROSS_PAIRS = [22, 23, 24, 25]  # pair indices holding local tiles 0..7
# DMA chunk sizes in PAIRS: small head pieces so the first pair lands fast
# despite queue fair-sharing, large mid chunks to keep the Sync engine's
# ~0.65us/trigger off the critical path
CHUNKS = [1, 1, 2, 8, 8, 4, 4, 2, 1, 1]
assert sum(CHUNKS) == NPAIR

# wmat slot layout (each slot [P, 2, MP]): pair p2 in 0..31 -> packed
# [w(ORDER[2p2]) | w(ORDER[2p2+1])] (U-pack for crossing pairs); 32..35 ->
# L-packs of crossing pairs; 36..43 -> per crossing tile j: [WL_j | WU_j]
NSLOT = NPAIR + 4 + TPC

_BASS_CACHE = {}


def _build_bass():
    import concourse.tile as tile
    import concourse.mybir as mybir
    from concourse import bacc

    DR = mybir.MatmulPerfMode.DoubleRow

    # Bacc (not raw Bass): its compile() runs generate_event_semaphores,
    # which splits multi-sem waits — TRN2 instructions hold at most one.
    nc = bacc.Bacc("TRN2")
    adj = nc.dram_tensor("adj", [P, NT, SLAB], mybir.dt.float8e4, kind="ExternalInput")
    wmat = nc.dram_tensor(
        "wmat", [P, 2 * NSLOT, MP], mybir.dt.float8e4, kind="ExternalInput"
    )
    masks = nc.dram_tensor(
        "masks", [P, 2 * SLAB], mybir.dt.float8e4, kind="ExternalInput"
    )
    stats = nc.dram_tensor("stats", [M, SLAB], mybir.dt.float32, kind="ExternalOutput")

    with tile.TileContext(nc) as tc:
        with (
            tc.tile_pool(name="singles", bufs=1) as singles,
            tc.tile_pool(name="diag", bufs=2 * TPC) as diag_pool,
            tc.tile_pool(name="psum", bufs=1, space="PSUM") as psum_pool,
        ):
            # weights first (first matmul needs them), then adjacency chunks;
            # masks slot in behind the first two chunks (needed mid-stream)
            wsb = singles.tile([P, 2 * NSLOT, MP], mybir.dt.float8e4)
            nc.sync.dma_start(out=wsb, in_=wmat[:, :, :])

            # all chunks are SBUF-resident (28KB/partition total) with no
            # pool reuse -> DMA triggers carry no reuse waits at all
            chunk_tiles = []
            pos = 0
            for i, sz in enumerate(CHUNKS):
                t = singles.tile(
                    [P, 2 * sz, SLAB], mybir.dt.float8e4,
                    tag=f"chunk{i}", name=f"chunk{i}",
                )
                chunk_tiles.append((t, pos, sz))
                nc.sync.dma_start(out=t, in_=adj[:, 2 * pos : 2 * (pos + sz), :])
                pos += sz
                if i == 1:
                    msb_raw = singles.tile([P, 2 * SLAB], mybir.dt.float8e4)
                    nc.sync.dma_start(out=msb_raw, in_=masks[:, :])

            # Re-produce the masks on DVE: the DVE TensorTensor ISA struct has
            # room for a single sync-wait, so the diag-mask multiplies must
            # only ever depend on DVE-produced operands (one self-sem wait).
            msb = singles.tile([P, 2 * SLAB], mybir.dt.float8e4)
            nc.vector.tensor_copy(msb, msb_raw)

            acc = psum_pool.tile([MP, SLAB], mybir.dt.float32, tag="acc", name="acc")

            def wpair(slot):  # [P, 2, MP] DoubleRow stationary pack
                return wsb[:, 2 * slot : 2 * slot + 2, :]

            def wone(slot, half):  # [P, MP] plain stationary
                return wsb[:, 2 * slot + half, :]

            # start=True zeroes the ENTIRE psum bank a matmul touches; SLAB
            # (448) fits one 512-col bank, so only the first matmul starts.
            state = {"started": False}

            def mm_dr(slot, t, k, a, b, stop=False):
                if a >= b:
                    return
                nc.tensor.matmul(
                    acc[:, a:b], wpair(slot), t[:, 2 * k : 2 * k + 2, a:b],
                    start=not state["started"], stop=stop,
                    perf_mode=DR, skip_group_check=True,
                )
                state["started"] = True

            def mm1(w, rhs2, out_a, out_b, stop=False):
                if out_a >= out_b:
                    return
                nc.tensor.matmul(
                    acc[:, out_a:out_b], w, rhs2,
                    start=not state["started"], stop=stop,
                    skip_group_check=True,
                )
                state["started"] = True

            for t, pos0, sz in chunk_tiles:
                for k in range(sz):
                    p2 = pos0 + k  # pair index
                    last = p2 == NPAIR - 1
                    if last:
                        # fine-grained tail: shortest latency from data
                        # arrival to final matmul (bounds 16-aligned)
                        bounds = [0, 112, 224, 336, SLAB]
                        for s, e in zip(bounds[:-1], bounds[1:]):
                            mm_dr(p2, t, k, s, e, stop=(e == SLAB))
                        continue
                    if p2 in CROSS_PAIRS:
                        q = p2 - CROSS_PAIRS[0]
                        j = 2 * q  # local crossing tiles j, j+1
                        zA, zB, zC, zD = j * W, (j + 1) * W, (j + 2) * W, SLAB
                        k0, k1 = 2 * k, 2 * k + 1
                        # zone A [0, zA): both tiles U -> U-pack DoubleRow
                        mm_dr(p2, t, k, 0, zA)
                        # zone B [zA, zB): tile j masked, tile j+1 full U
                        mlo0 = diag_pool.tile([P, W], mybir.dt.float8e4)
                        nc.vector.tensor_mul(mlo0, t[:, k0, zA:zB], msb[:, zA:zB])
                        mup0 = diag_pool.tile([P, W], mybir.dt.float8e4)
                        nc.vector.tensor_mul(
                            mup0, t[:, k0, zA:zB], msb[:, SLAB + zA : SLAB + zB]
                        )
                        mm1(wone(36 + j, 0), mlo0, zA, zB)
                        mm1(wone(36 + j, 1), mup0, zA, zB)
                        mm1(wone(36 + j + 1, 1), t[:, k1, zA:zB], zA, zB)
                        # zone C [zB, zC): tile j full L, tile j+1 masked
                        mlo1 = diag_pool.tile([P, W], mybir.dt.float8e4)
                        nc.vector.tensor_mul(mlo1, t[:, k1, zB:zC], msb[:, zB:zC])
                        mup1 = diag_pool.tile([P, W], mybir.dt.float8e4)
                        nc.vector.tensor_mul(
                            mup1, t[:, k1, zB:zC], msb[:, SLAB + zB : SLAB + zC]
                        )
                        mm1(wone(36 + j, 0), t[:, k0, zB:zC], zB, zC)
                        mm1(wone(36 + j + 1, 0), mlo1, zB, zC)
                        mm1(wone(36 + j + 1, 1), mup1, zB, zC)
                        # zone D [zC, 448): both tiles L -> L-pack DoubleRow
                        mm_dr(NPAIR + q, t, k, zC, zD)
                    else:
                        mm_dr(p2, t, k, 0, SLAB)

            # copy-out tail split across ACT and DVE so the halves run in
            # parallel right after the final matmul
            out_sb = singles.tile([M, SLAB], mybir.dt.float32)
            half = SLAB // 2
            nc.scalar.copy(out_sb[:, 0:half], acc[0:M, 0:half])
            nc.vector.tensor_copy(out_sb[:, half:], acc[0:M, half:])
            nc.sync.dma_start(out=stats[:, :], in_=out_sb[:, :])

    nc.compile()
    return nc


def _split_fp8(v):
    hi = v.astype(FP8)
    lo = (v - hi.astype(np.float64)).astype(FP8)
    return hi, lo


def _host_weights(outputs, targets):
    """Per-row weight table Wside [N, 6] fp8 (hi/lo split pairs)."""
    out = np.asarray(outputs, np.float64).reshape(-1)
    pos = (np.asarray(targets).reshape(-1) != 0).astype(np.float64)
    pl_hi, pl_lo = _split_fp8(pos * out)
    e_hi, e_lo = _split_fp8(np.exp(out))
    return np.stack(
        [np.ones(N, FP8), pos.astype(FP8), pl_hi, pl_lo, e_hi, e_lo], axis=1
    ).astype(FP8)  # [N, 6]


def _tile_weights(wside, core):
    """Per local tile j: (wl[128, MP], wu[128, MP]) fp8, zero-padded.

    wl has the L stats in rows 0:6, wu the U stats in rows 6:12, matching
    the psum layout; for non-crossing tiles only the relevant one is used.
    """
    wl = np.zeros((NT, P, MP), FP8)
    wu = np.zeros((NT, P, MP), FP8)
    for j in range(NT):
        t = (TPC * core + j) % NT
        rows = wside[t * P : (t + 1) * P, :]  # [128, 6]
        wl[j, :, 0:NW] = rows
        wu[j, :, NW:M] = rows
    return wl, wu


def _build_wmat(wside, core):
    """Slot-packed stationary weights [P, 2*NSLOT, MP] fp8 (see layout)."""
    wl, wu = _tile_weights(wside, core)
    w = np.zeros((P, 2 * NSLOT, MP), FP8)

    def tile_w(j):
        # routing for a full (non-crossing) tile: U if its absolute tile
        # index is above the slab's column range, else L (wrapped rows)
        return wu[j] if j < NT - TPC * core else wl[j]

    for p2 in range(NPAIR):
        j0, j1 = ORDER[2 * p2], ORDER[2 * p2 + 1]
        if p2 in CROSS_PAIRS:
            w[:, 2 * p2] = wu[j0]      # U-pack (zone A)
            w[:, 2 * p2 + 1] = wu[j1]
        else:
            w[:, 2 * p2] = tile_w(j0)
            w[:, 2 * p2 + 1] = tile_w(j1)
    for q in range(4):  # L-packs of crossing pairs (zone D)
        w[:, 2 * (NPAIR + q)] = wl[2 * q]
        w[:, 2 * (NPAIR + q) + 1] = wl[2 * q + 1]
    for j in range(TPC):  # plain crossing variants
        w[:, 2 * (36 + j)] = wl[j]
        w[:, 2 * (36 + j) + 1] = wu[j]
    return np.ascontiguousarray(w)


def _plan_columns(idx_node):
    """Distinct query columns -> per-core padded slot plan + host overflow."""
    idx = np.asarray(idx_node).reshape(-1).astype(np.int64)
    dist, mult = np.unique(idx, return_counts=True)
    plan = [[[] for _ in range(TPC)] for _ in range(NCORES)]
    overflow = []
    for c, m in zip(dist, mult):
        d, g = int(c) // CRANGE, (int(c) % CRANGE) // P
        if len(plan[d][g]) < W:
            plan[d][g].append((int(c), int(m)))
        else:
            overflow.append((int(c), int(m)))
    return plan, overflow


def _build_inputs(node_adj, wside, plan):
    """Per-core in_maps: tile-major rotated fp8 slab, weights, step masks."""
    node_adj = np.asarray(node_adj)
    in_maps = []
    ri = np.arange(P)[:, None]  # [128,1]
    for d in range(NCORES):
        slab = np.zeros((N, SLAB), FP8)
        maskL = np.zeros((P, SLAB), FP8)
        maskU = np.zeros((P, SLAB), FP8)
        c0 = CRANGE * d
        for g in range(TPC):
            entries = plan[d][g]
            if not entries:
                continue
            cols = np.array([c for c, _ in entries], np.int64)
            sl = slice(g * W, g * W + len(cols))
            gathered = (node_adj[:, cols] != 0).astype(FP8)
            # rotate rows: local row rho = (abs_row - CRANGE*d) mod N
            slab[:, sl] = np.concatenate([gathered[c0:], gathered[:c0]], axis=0)
            # strict step masks vs the crossing tile's absolute row base;
            # strictness zeroes the diagonal on both sides
            base = c0 + g * P
            maskL[:, sl] = (base + ri < cols[None, :]).astype(FP8)
            maskU[:, sl] = (base + ri > cols[None, :]).astype(FP8)
        # tile-major DRAM layout in processing order
        adjT = np.zeros((P, NT, SLAB), FP8)
        for p, j in enumerate(ORDER):
            adjT[:, p, :] = slab[j * P : (j + 1) * P, :]
        in_maps.append(
            {
                "adj": np.ascontiguousarray(adjT),
                "wmat": _build_wmat(wside, d),
                "masks": np.ascontiguousarray(np.concatenate([maskL, maskU], axis=1)),
            }
        )
    return in_maps


def _side_contrib(x):
    cnt, poscnt = x[0], x[1]
    poslogit = x[2] + x[3]
    sumexp = x[4] + x[5]
    valid = (cnt > 0.5) & (np.abs(poscnt - 1.0) < 0.25)
    lse = np.log(np.where(valid, np.maximum(sumexp, 1e-300), 1.0))
    return np.where(valid, (lse - poslogit) / np.maximum(cnt, 1.0), 0.0)


def _combine(stats_list, plan, overflow, outputs, targets, node_adj):
    """stats_list: per-core [12, SLAB] f32 -> scalar loss (f64 math)."""
    total = 0.0
    for d in range(NCORES):
        x = np.asarray(stats_list[d], np.float64)
        contrib = _side_contrib(x[0:NW]) + _side_contrib(x[NW:M])
        for g in range(TPC):
            for i, (_, m) in enumerate(plan[d][g]):
                total += m * contrib[g * W + i]
    if overflow:
        out = np.asarray(outputs, np.float64).reshape(-1)
        pos = np.asarray(targets).reshape(-1) != 0
        rows = np.arange(N)
        for c, m in overflow:
            col = (np.asarray(node_adj[:, c]).reshape(-1) != 0) & (rows != c)
            for mask in (col & (rows < c), col & (rows >= c)):
                cnt = int(mask.sum())
                pcnt = int((mask & pos).sum())
                if cnt > 0 and pcnt == 1:
                    lse = np.log(np.exp(out[mask]).sum())
                    pl = out[mask & pos].sum()
                    total += m * (lse - pl) / cnt
    return np.float32(total)


def _ensure_axon_hooks_stub():
    """bass_utils imports antenv.axon_hooks when tracing is requested via
    env; the module is absent on some images. Provide a no-op stub so the
    import never crashes (hook=None -> bass_utils skips tracing)."""
    import sys
    import types

    try:
        import antenv.axon_hooks  # noqa: F401
    except ImportError:
        mod = types.ModuleType("antenv.axon_hooks")
        state = {"hook": None}
        mod.set_axon_ntff_profile_hook = lambda h: state.__setitem__("hook", h)
        mod.get_axon_ntff_profile_hook = lambda: state["hook"]
        sys.modules["antenv.axon_hooks"] = mod


def _device_stats(in_maps):
    _ensure_axon_hooks_stub()
    from concourse.bass_utils import run_bass_kernel_spmd

    if "nc" not in _BASS_CACHE:
        _BASS_CACHE["nc"] = _build_bass()
    last_exc = None
    for attempt in range(4):
        try:
            res = run_bass_kernel_spmd(
                _BASS_CACHE["nc"], in_maps, core_ids=list(range(NCORES))
            )
            return [r["stats"] for r in res.results]
        except Exception as e:  # transient NRT/accelerator hiccups
            last_exc = e
            try:
                # a fresh PJRT client usually recovers a transiently
                # "unrecoverable" accelerator; mirrors a process restart
                import jax
                import jax.extend.backend as _jeb

                jax.clear_caches()
                _jeb.clear_backends()
            except Exception:
                pass
            import time

            time.sleep(2.0 * (attempt + 1))
    raise last_exc


def _sim_stats(in_maps):
    """Numpy emulation of the device kernel (same inputs incl. fp8
    quantization and wmat slot packing), for logic + precision validation."""
    outs = []
    for im in in_maps:
        adjT = im["adj"].astype(np.float32)           # [P, NT, SLAB]
        w = im["wmat"].astype(np.float32)             # [P, 2*NSLOT, MP]
        msk = im["masks"].astype(np.float32)
        acc = np.zeros((MP, SLAB), np.float32)
        for p2 in range(NPAIR):
            a0, a1 = adjT[:, 2 * p2, :], adjT[:, 2 * p2 + 1, :]
            if p2 in CROSS_PAIRS:
                q = p2 - CROSS_PAIRS[0]
                j = 2 * q
                zA, zB, zC = j * W, (j + 1) * W, (j + 2) * W
                acc[:, :zA] += w[:, 2 * p2].T @ a0[:, :zA]
                acc[:, :zA] += w[:, 2 * p2 + 1].T @ a1[:, :zA]
                acc[:, zA:zB] += w[:, 2 * (36 + j)].T @ (a0[:, zA:zB] * msk[:, zA:zB])
                acc[:, zA:zB] += w[:, 2 * (36 + j) + 1].T @ (
                    a0[:, zA:zB] * msk[:, SLAB + zA : SLAB + zB]
                )
                acc[:, zA:zB] += w[:, 2 * (36 + j + 1) + 1].T @ a1[:, zA:zB]
                acc[:, zB:zC] += w[:, 2 * (36 + j)].T @ a0[:, zB:zC]
                acc[:, zB:zC] += w[:, 2 * (36 + j + 1)].T @ (
                    a1[:, zB:zC] * msk[:, zB:zC]
                )
                acc[:, zB:zC] += w[:, 2 * (36 + j + 1) + 1].T @ (
                    a1[:, zB:zC] * msk[:, SLAB + zB : SLAB + zC]
                )
                acc[:, zC:] += w[:, 2 * (NPAIR + q)].T @ a0[:, zC:]
                acc[:, zC:] += w[:, 2 * (NPAIR + q) + 1].T @ a1[:, zC:]
            else:
                acc += w[:, 2 * p2].T @ a0
                acc += w[:, 2 * p2 + 1].T @ a1
        outs.append(acc[0:M])
    return outs


def prepare(outputs, targets, node_adj, idx_node):
    wside = _host_weights(outputs, targets)
    plan, overflow = _plan_columns(idx_node)
    in_maps = _build_inputs(node_adj, wside, plan)
    return in_maps, plan, overflow


def kernel(outputs, targets, node_adj, idx_node, _simulate=False):
    in_maps, plan, overflow = prepare(outputs, targets, node_adj, idx_node)
    stats = _sim_stats(in_maps) if _simulate else _device_stats(in_maps)
    return _combine(stats, plan, overflow, outputs, targets, node_adj)
